# revision 1
# baseline (speedup 1.0000x reference)
"""Additive (Bahdanau) attention on Trainium2, data-parallel over batch on 8 NeuronCores.

Math (per batch b):
    qp = queries @ W_q                     [Tq, H]
    kp = keys @ W_k + b                    [Tk, H]
    scores[q,k] = sum_h v[h] * tanh(qp[q,h] + kp[k,h])
    masked softmax over k (k < seq_len[b]), then out = align @ keys.

Design (per core, 4 batch "slots" with compile-time key-lengths L_slots):
  - host packs keys||ones||maskbias into one "keysx" input so a single DMA per
    k-chunk provides matmul rhs, softmax mask bias and values.
  - keys/queries transposed on PE (identity matmul); projections on PE with the
    b bias folded in via the ones column / W_k||b const rows.
  - kpb duplicated across both 64-partition halves -> kpb2 [128=2h, L].
  - S[h2, j*L+k] = kpb2 + qp2[:, j] per query-pair j via DVE tensor_scalar adds
    (f32 2x port mode), tanh on ACT in two big ops per slot.
  - scores^T[k, q] via PE matmuls: lhsT = tanh tile [128h2, <=128k] stationary,
    rhs = v2blk [128, 2] -> psum [k, 2q] per query pair (block-diagonal v gives
    both queries of a pair in one matmul).
  - exp on ACT from PSUM with per-partition bias = 0/-30000 mask column.
  - final: out_un[q, h] | rowsum = E-chunks (lhsT) @ [keys || ones] (rhs),
    PSUM-accumulated over k-chunks; divide via DVE reciprocal + scale.

Batches are sorted by seq_len and dealt so each core gets one batch per slot
rank; slot k-length = max over the 8 batches of that rank (padded to 8). All
cores run the identical program on different data (SPMD).
"""

import sys

_REPO = "/opt/trn_rl_repo"
if _REPO not in sys.path:
    sys.path.insert(0, _REPO)

import numpy as np

from concourse import bacc, tile
import concourse.mybir as mybir
from concourse import bass_utils

B, TQ, TK, H = 32, 64, 256, 64
NCORES = 8
SLOTS = 4
F32 = mybir.dt.float32
BF16 = mybir.dt.bfloat16
TANH = mybir.ActivationFunctionType.Tanh
EXP = mybir.ActivationFunctionType.Exp
MASK_NEG = -30000.0
KX = H + 2  # keys | ones | maskbias

_prog_cache: dict = {}


def _roundup(x, m):
    return ((x + m - 1) // m) * m


def _chunks(L):
    out, off = [], 0
    while off < L:
        w = min(128, L - off)
        out.append((off, w))
        off += w
    return out


def _build(L_slots):
    nc = bacc.Bacc(
        "TRN2",
        target_bir_lowering=False,
        debug=False,
        enable_asserts=False,
        num_devices=NCORES,
    )
    kx_d = nc.dram_tensor("keysx", [128, 8 * KX], F32, kind="ExternalInput").ap()
    cx_d = nc.dram_tensor("cx", [128, 452], F32, kind="ExternalInput").ap()
    o_d = nc.dram_tensor("out", [SLOTS, TQ, H], F32, kind="ExternalOutput").ap()

    with tile.TileContext(nc) as tc:
        with (
            tc.tile_pool(name="const", bufs=1) as cpool,
            tc.tile_pool(name="qpool", bufs=1) as qpool,
            tc.tile_pool(name="kpool", bufs=2) as kpool,
            tc.tile_pool(name="wpool", bufs=3) as wpool,
            tc.tile_pool(name="spool", bufs=2) as spool,
            tc.tile_pool(name="tpp", bufs=2, space="PSUM") as tpp,
            tc.tile_pool(name="prj", bufs=2, space="PSUM") as prj,
            tc.tile_pool(name="scp", bufs=2, space="PSUM") as scp,
            tc.tile_pool(name="oup", bufs=2, space="PSUM") as oup,
        ):
            # tiny activation up front so the ACT table set loads early
            scr = cpool.tile([1, 2], F32, name="scr", tag="scr")
            nc.vector.memset(scr, 0.0)
            nc.scalar.activation(scr, scr, TANH)

            # ---- prefetch everything up front in two big DMAs.
            cx_sb = cpool.tile([128, 452], F32, name="cx_sb", tag="cx")
            nc.sync.dma_start(out=cx_sb, in_=cx_d)
            qx_sb = cx_sb[:, 0:130]
            id_sb = cx_sb[:, 130:258]
            wk2_sb = cx_sb[0:H, 258:386]
            b2_col = cx_sb[:, 386:387]
            wq_sb = cx_sb[0:H, 387:451]
            v2_sb = cpool.tile([128, 2], BF16, name="v2_sb", tag="v2")
            nc.vector.tensor_copy(v2_sb, qx_sb[:, 128:130])

            all_chs = {s: _chunks(L_slots[s]) for s in range(SLOTS)}
            knat_all = kpool.tile(
                [128, 8 * KX], F32, name="knat_all", tag="knat_all", bufs=1
            )
            nc.sync.dma_start(out=knat_all, in_=kx_d)
            knat = {}
            for s in range(SLOTS):
                for ci, (off, w) in enumerate(all_chs[s]):
                    idx = 2 * s + ci
                    knat[(s, ci)] = knat_all[:, idx * KX : (idx + 1) * KX]

            # queries: transpose + project, two slots at a time
            qp2g = [None, None]

            def qwork(g):
                qT_ps = tpp.tile([H, 128], F32, name=f"qTps{g}", tag="tp")
                nc.tensor.transpose(qT_ps, qx_sb[:, 64 * g : 64 * g + 64], id_sb)
                qT_sb = wpool.tile([H, 128], F32, name=f"qTsb{g}", tag="qT")
                nc.vector.tensor_copy(qT_sb, qT_ps)
                qpT_ps = prj.tile([H, 128], F32, name=f"qpTps{g}", tag="prj")
                nc.tensor.matmul(qpT_ps, lhsT=wq_sb, rhs=qT_sb)
                # qp2rep[0:64, 2j] = qp2rep[0:64, 2j+1] = qpT[:, 2j]
                # qp2rep[64:128, 2j] = qp2rep[64:128, 2j+1] = qpT[:, 2j+1]
                qp2 = qpool.tile([128, 128], BF16, name=f"qp2_{g}", tag=f"qp2_{g}")
                nc.vector.tensor_copy(
                    qp2[0:64, :].rearrange("c (j two) -> c j two", two=2),
                    qpT_ps[:, 0:128:2].unsqueeze(2).broadcast_to([H, 64, 2]),
                )
                nc.vector.tensor_copy(
                    qp2[64:128, :].rearrange("c (j two) -> c j two", two=2),
                    qpT_ps[:, 1:128:2].unsqueeze(2).broadcast_to([H, 64, 2]),
                )
                qp2g[g] = qp2

            keysTs, kpb2s = {}, {}

            def prep(s):
                L = L_slots[s]
                chs = all_chs[s]
                keysT = kpool.tile([H, TK], F32, name=f"keysT{s}", tag="keysT")
                for ci, (off, w) in enumerate(chs):
                    kT_ps = tpp.tile([H, 128], F32, name=f"kTps{s}_{ci}", tag="tp")
                    nc.tensor.transpose(
                        kT_ps[0:H, 0:w], knat[(s, ci)][0:w, 0:H], id_sb[0:w, 0:w]
                    )
                    nc.vector.tensor_copy(keysT[0:H, off : off + w], kT_ps[0:H, 0:w])
                kpT_ps = prj.tile([128, TK], F32, name=f"kpTps{s}", tag="prj")
                nc.tensor.matmul(kpT_ps[:, 0:L], lhsT=wk2_sb, rhs=keysT[:, 0:L])
                kpb2 = wpool.tile([128, TK], BF16, name=f"kpb2_{s}", tag="kpb2")
                nc.vector.tensor_scalar_add(kpb2[:, 0:L], kpT_ps[:, 0:L], b2_col)
                kpb2s[s] = kpb2

            def compute(s):
                L = L_slots[s]
                chs = all_chs[s]
                nch = len(chs)
                kpb2 = kpb2s[s]
                qp2 = qp2g[s // 2]
                qoff = 64 * (s % 2)
                S_all = spool.tile([128, 32 * L], BF16, name=f"S{s}", tag="S")
                for j0, j1 in ((0, 8), (8, 20), (20, 32)) if s == 0 else ((0, 16), (16, 32)):
                    nj = j1 - j0
                    in0 = (
                        kpb2[:, 0:L]
                        .rearrange("c (k two) -> c k two", two=2)
                        .unsqueeze(1)
                        .broadcast_to([128, nj, L // 2, 2])
                    )
                    in1 = (
                        qp2[:, qoff + 2 * j0 : qoff + 2 * j1]
                        .rearrange("c (j two) -> c j two", two=2)
                        .unsqueeze(2)
                        .broadcast_to([128, nj, L // 2, 2])
                    )
                    s_out = S_all[:, j0 * L : j1 * L].rearrange(
                        "c (j k two) -> c j k two", two=2, k=L // 2
                    )
                    nc.vector.tensor_add(s_out, in0, in1)
                S_tanh = spool.tile([128, 32 * L], BF16, name=f"T{s}", tag="T")
                for t0, t1 in ((0, 8), (8, 20), (20, 32)) if s == 0 else ((0, 16), (16, 32)):
                    nc.scalar.activation(
                        S_tanh[:, t0 * L : t1 * L], S_all[:, t0 * L : t1 * L], TANH
                    )
                out_ps = oup.tile([TQ, H + 1], F32, name=f"ops{s}", tag="ou")
                for ci, (off, w) in enumerate(chs):
                    kn = knat[(s, ci)]
                    sc_ps = scp.tile([128, TQ], F32, name=f"sc{s}_{ci}", tag="sc")
                    for j in range(32):
                        nc.tensor.matmul(
                            sc_ps[0:w, 2 * j : 2 * j + 2],
                            lhsT=S_tanh[:, j * L + off : j * L + off + w],
                            rhs=v2_sb,
                            start=True,
                            stop=True,
                        )
                    E = wpool.tile([128, TQ], F32, name=f"E{s}_{ci}", tag=f"E{ci}")
                    nc.scalar.activation(
                        E[0:w, :], sc_ps[0:w, :], EXP, bias=kn[0:w, H + 1 : H + 2]
                    )
                    nc.tensor.matmul(
                        out_ps,
                        lhsT=E[0:w, 0:TQ],
                        rhs=kn[0:w, 0 : H + 1],
                        start=(ci == 0),
                        stop=(ci == nch - 1),
                    )
                recip = wpool.tile([TQ, 1], F32, name=f"rc{s}", tag="rc")
                nc.vector.reciprocal(recip, out_ps[:, H : H + 1])
                out_sb = wpool.tile([TQ, H], F32, name=f"osb{s}", tag="osb")
                nc.vector.tensor_scalar_mul(out_sb, out_ps[:, 0:H], recip)
                nc.sync.dma_start(out=o_d[s], in_=out_sb)

            # critical path to first TANH: qp2(pair0) + kpb2(slot0); defer pair-1
            # query work until after compute(0) is issued.
            qwork(0)
            prep(0)
            prep(1)
            compute(0)
            qwork(1)
            prep(2)
            compute(1)
            prep(3)
            compute(2)
            compute(3)

    nc.compile()
    return nc


def _get_prog(L_slots):
    if L_slots not in _prog_cache:
        _prog_cache[L_slots] = _build(L_slots)
    return _prog_cache[L_slots]


def _plan(seq_len_flat):
    sl = np.asarray(seq_len_flat).reshape(-1).astype(np.int64)
    order = np.argsort(-sl, kind="stable")
    assign = np.zeros((NCORES, SLOTS), dtype=np.int64)
    L_slots = []
    for s in range(SLOTS):
        grp = order[NCORES * s : NCORES * (s + 1)]
        assign[:, s] = grp
        L = int(max(1, sl[grp].max()))
        L_slots.append(min(TK, _roundup(L, 8)))
    return tuple(L_slots), assign, sl


def _make_in_maps(queries, keys, sl, assign, W_q, W_k, v, b):
    vv = np.asarray(v, dtype=np.float32).reshape(-1)
    bb = np.asarray(b, np.float32).reshape(-1)
    base = np.zeros((128, 452), np.float32)
    base[:, 130:258] = np.eye(128, dtype=np.float32)
    base[0:H, 258:322] = W_k
    base[0:H, 322:386] = W_k
    base[0:H, 386] = bb
    base[H:128, 386] = bb
    base[0:H, 387:451] = W_q

    in_maps = []
    for c in range(NCORES):
        bidx = assign[c]
        cx = base.copy()
        cx[:, 0:64] = queries[bidx[0:2]].reshape(128, H)
        cx[:, 64:128] = queries[bidx[2:4]].reshape(128, H)
        cx[0:64, 128] = vv
        cx[64:128, 129] = vv
        keysx = np.zeros((SLOTS, TK, KX), np.float32)
        keysx[:, :, 0:H] = keys[bidx]
        keysx[:, :, H] = 1.0
        for s_i, b_i in enumerate(bidx):
            keysx[s_i, sl[b_i] :, H + 1] = MASK_NEG
        # pre-transpose on host so the device DMA is fully contiguous:
        # [s, ci*128+p, x] -> [p, (2s+ci)*KX + x]
        keysx = np.ascontiguousarray(
            keysx.reshape(SLOTS, 2, 128, KX).transpose(2, 0, 1, 3).reshape(128, 8 * KX)
        )
        in_maps.append(
            {
                "keysx": keysx,
                "cx": cx,
            }
        )
    return in_maps


def _run_spmd(nc, in_maps, trace=False, trace_kwargs=None):
    from concourse.bass_interp import get_hw_module

    old = nc.m
    nc.m = get_hw_module(nc.m)
    try:
        res = bass_utils.run_bass_kernel_spmd(
            nc,
            in_maps,
            core_ids=list(range(NCORES)),
            trace=trace,
            **(trace_kwargs or {}),
        )
    finally:
        nc.m = old
    return res


def kernel(queries, keys, seq_len, W_q, W_k, v, b, _trace=False):
    queries = np.asarray(queries, dtype=np.float32)
    keys = np.asarray(keys, dtype=np.float32)
    L_slots, assign, sl = _plan(seq_len)
    nc = _get_prog(L_slots)
    in_maps = _make_in_maps(queries, keys, sl, assign, W_q, W_k, v, b)
    res = _run_spmd(nc, in_maps, trace=_trace)
    out = np.zeros((B, TQ, H), np.float32)
    for c in range(NCORES):
        o = res.results[c]["out"]
        for s_i, b_i in enumerate(assign[c]):
            out[b_i] = o[s_i]
    # seq_len==0 -> reference softmax degenerates to uniform over all keys
    # (all positions masked to the same NEG_PAD). Never occurs with the fixed
    # seed, but keep the exact semantics.
    for b_i in np.nonzero(sl == 0)[0]:
        out[b_i] = keys[b_i].mean(axis=0, keepdims=True)
    if _trace:
        kernel._last_results = res
    return out



# revision 15
# speedup vs baseline: 1.0601x; 1.0601x over previous
"""Additive (Bahdanau) attention on Trainium2, data-parallel over batch on 8 NeuronCores.

Math (per batch b):
    qp = queries @ W_q                     [Tq, H]
    kp = keys @ W_k                        [Tk, H]
    scores[q,k] = sum_h v[h] * tanh(qp[q,h] + kp[k,h] + b[h])
    masked softmax over k (k < seq_len[b]), then out = align @ keys.

Design (per core, 4 batch "slots" with compile-time key-lengths L_slots,
ordered [3rd-longest, longest, 2nd-longest, shortest] so the pipeline fills
fast and drains on the smallest slot):
  - ALL host-side layout prep: transposed bf16 keys (ktall), natural
    mask-zeroed bf16 keys||maskones chunks (knall), and a const tensor cx
    holding W_q block-diag, [W_k|W_k], b, v2 and the pair-duplicated
    transposed queries (qTx) - so the device does zero transposes.
  - qp2 [128, 128] per slot-pair in ONE matmul: lhsT = blockdiag(W_q, W_q),
    rhs = qTx (host-arranged so column n yields query 2*(n%64//2)+half),
    then one DVE tensor_scalar_add folds the +b bias (bf16 out).
  - kp per slot: one matmul (lhsT=[W_k|W_k] -> both 64-partition halves),
    one DVE copy PSUM->SBUF bf16.
  - S[h2, j*L+k] = kpb2 + qp2[:, 2j+par] via DVE tensor_tensor adds in bf16
    (2x mode), tanh on ACT (the roofline: 32*sum(L) lanes-cycles @1.2GHz).
  - scores^T[k, 2j+t] via PE: lhsT = S_tanh[:, j*L+off:+w] stationary,
    rhs = v2 [128, 2] -> psum[w, 2j:2j+2]; both k-chunks of a slot land in
    one PSUM tile [128, 64*nch].
  - ONE exp per slot (no bias needed: masking lives in knall's zeroed rows),
    bf16 out -> E feeds the final matmuls directly.
  - out_un[q, h]|rowsum: PSUM-accumulated E-chunks @ knall chunks; DVE
    reciprocal + scale, DMA out.

Batches are sorted by seq_len and dealt so each core gets one batch per slot
rank; slot k-length = max over the 8 batches of that rank (padded to 8). All
cores run the identical program on different data (SPMD).
"""

import sys

_REPO = "/opt/trn_rl_repo"
if _REPO not in sys.path:
    sys.path.insert(0, _REPO)

import numpy as np

try:
    from ml_dtypes import bfloat16 as np_bf16
except ImportError:  # pragma: no cover
    import jax.numpy as _jnp

    np_bf16 = _jnp.bfloat16

from concourse import bacc, tile
import concourse.mybir as mybir
from concourse import bass_utils

B, TQ, TK, H = 32, 64, 256, 64
NCORES = 8
SLOTS = 4
F32 = mybir.dt.float32
BF16 = mybir.dt.bfloat16
TANH = mybir.ActivationFunctionType.Tanh
EXP = mybir.ActivationFunctionType.Exp
KXC = 66  # keys | maskones | pad, per 128-row chunk of knall

_prog_cache: dict = {}


def _roundup(x, m):
    return ((x + m - 1) // m) * m


def _nch(L):
    return (L + 127) // 128


_SLOT_SPLITS = [
    [(0, 8), (8, 20), (20, 32)],  # slot 0: fine start for pipeline fill
    [(0, 12), (12, 24), (24, 32)],  # slot 1 (longest)
    [(0, 16), (16, 32)],
    [(0, 16), (16, 32)],
]


def _build(L_slots):
    nc = bacc.Bacc(
        "TRN2",
        target_bir_lowering=False,
        debug=False,
        enable_asserts=False,
        num_devices=NCORES,
    )
    nchs = [_nch(L) for L in L_slots]
    KT = sum(L_slots)
    KN = sum(nchs) * KXC
    CX = 260 + 256  # wq_blk 128 | wk2 128 | b2 1 | v2 2 | pad 1 | qTx 2*128

    cx_d = nc.dram_tensor("cx", [128, CX], BF16, kind="ExternalInput").ap()
    kt_d = nc.dram_tensor("ktall", [H, KT], BF16, kind="ExternalInput").ap()
    kn_d = nc.dram_tensor("knall", [128, KN], BF16, kind="ExternalInput").ap()
    o_d = nc.dram_tensor("out", [SLOTS, TQ, H], F32, kind="ExternalOutput").ap()

    ktoff = [sum(L_slots[:s]) for s in range(SLOTS)]
    knoff = [sum(nchs[:s]) * KXC for s in range(SLOTS)]

    with tile.TileContext(nc) as tc:
        with (
            tc.tile_pool(name="const", bufs=1) as cpool,
            tc.tile_pool(name="qpool", bufs=2) as qpool,
            tc.tile_pool(name="kpb", bufs=2) as kpool,
            tc.tile_pool(name="spool", bufs=2) as spool,
            tc.tile_pool(name="epool", bufs=2) as epool,
            tc.tile_pool(name="opool", bufs=2) as opool,
            tc.tile_pool(name="qpp", bufs=2, space="PSUM") as qpp,
            tc.tile_pool(name="kpp", bufs=2, space="PSUM") as kpp,
            tc.tile_pool(name="scp", bufs=2, space="PSUM") as scp,
            tc.tile_pool(name="oup", bufs=2, space="PSUM") as oup,
        ):
            # tiny activation up front so the ACT table set loads early
            scr = cpool.tile([1, 2], F32, name="scr", tag="scr")
            nc.vector.memset(scr, 0.0)
            nc.scalar.activation(scr, scr, TANH)

            # ---- prefetch everything in three DMAs (cx first: needed first)
            cx_sb = cpool.tile([128, CX], BF16, name="cx_sb", tag="cx")
            nc.sync.dma_start(out=cx_sb, in_=cx_d)
            kt_sb = cpool.tile([H, KT], BF16, name="kt_sb", tag="kt")
            nc.sync.dma_start(out=kt_sb, in_=kt_d)
            kn_sb = cpool.tile([128, KN], BF16, name="kn_sb", tag="kn")
            nc.sync.dma_start(out=kn_sb, in_=kn_d)

            wqb_sb = cx_sb[:, 0:128]
            wk2_sb = cx_sb[0:H, 128:256]
            b2_col = cpool.tile([128, 1], F32, name="b2f", tag="b2f")
            nc.vector.tensor_copy(b2_col, cx_sb[:, 256:257])
            v2_sb = cx_sb[:, 257:259]
            qtx = [cx_sb[:, 260:388], cx_sb[:, 388:516]]

            # ---- projections: all PE work up front (cold engine, tiny)
            qp2_sb = [None, None]

            def qproj_mm(g):
                ps = qpp.tile([128, 128], F32, name=f"qpps{g}", tag="qp")
                nc.tensor.matmul(ps, lhsT=wqb_sb, rhs=qtx[g])
                return ps

            def qproj_dve(g, ps):
                qp2 = qpool.tile([128, 128], BF16, name=f"qp2_{g}", tag="qp2")
                nc.vector.tensor_scalar_add(qp2, ps, b2_col)
                qp2_sb[g] = qp2

            kpb2s = {}

            Lmax = max(L_slots)
            nchmax = max(nchs)

            def kproj_mm(s):
                L = L_slots[s]
                ps = kpp.tile([128, Lmax], F32, name=f"kpps{s}", tag="kp")
                nc.tensor.matmul(
                    ps[:, 0:L], lhsT=wk2_sb, rhs=kt_sb[:, ktoff[s] : ktoff[s] + L]
                )
                return ps

            def kproj_dve(s, ps):
                L = L_slots[s]
                kpb2 = kpool.tile([128, Lmax], BF16, name=f"kpb2_{s}", tag="kpb2")
                nc.vector.tensor_copy(kpb2[:, 0:L], ps[:, 0:L])
                kpb2s[s] = kpb2

            qps0 = qproj_mm(0)
            qproj_dve(0, qps0)
            kps0 = kproj_mm(0)
            kproj_dve(0, kps0)
            kps1 = kproj_mm(1)
            qps1 = qproj_mm(1)

            # ---- per-slot stages
            S_alls, S_tanhs, sc_pss, E_sbs, out_pss = {}, {}, {}, {}, {}

            def adds(s, j0, j1):
                L = L_slots[s]
                if s not in S_alls:
                    S_alls[s] = spool.tile(
                        [128, 32 * Lmax], BF16, name=f"S{s}", tag="S"
                    )
                S_all = S_alls[s]
                kpb2 = kpb2s[s]
                qp2 = qp2_sb[s // 2]
                qoff = 64 * (s % 2)
                nj = j1 - j0
                in0 = (
                    kpb2[:, 0:L]
                    .rearrange("c (k two) -> c k two", two=2)
                    .unsqueeze(1)
                    .broadcast_to([128, nj, L // 2, 2])
                )
                in1 = (
                    qp2[:, qoff + 2 * j0 : qoff + 2 * j1]
                    .rearrange("c (j two) -> c j two", two=2)
                    .unsqueeze(2)
                    .broadcast_to([128, nj, L // 2, 2])
                )
                s_out = S_all[:, j0 * L : j1 * L].rearrange(
                    "c (j k two) -> c j k two", two=2, k=L // 2
                )
                nc.vector.tensor_add(s_out, in0, in1)

            def tanh(s, j0, j1):
                L = L_slots[s]
                if s not in S_tanhs:
                    S_tanhs[s] = spool.tile(
                        [128, 32 * Lmax], BF16, name=f"T{s}", tag="T"
                    )
                nc.scalar.activation(
                    S_tanhs[s][:, j0 * L : j1 * L],
                    S_alls[s][:, j0 * L : j1 * L],
                    TANH,
                )

            def scores(s):
                L = L_slots[s]
                nch = nchs[s]
                S_tanh = S_tanhs[s]
                sc_ps = scp.tile(
                    [128, 64 * nchmax], F32, name=f"sc{s}", tag="sc"
                )
                sc_pss[s] = sc_ps
                wl = L - 128 * (nch - 1)
                if nch > 1 and wl < 128:
                    # the one-exp-per-slot reads the full tile; pre-zero the
                    # last chunk's columns (its matmuls then overwrite rows
                    # [0:wl], leaving the tail rows initialized)
                    nc.vector.memset(sc_ps[:, 64 * (nch - 1) : 64 * nch], 0.0)
                for ci in range(nch):
                    off = 128 * ci
                    w = min(128, L - off)
                    for j in range(32):
                        nc.tensor.matmul(
                            sc_ps[0:w, 64 * ci + 2 * j : 64 * ci + 2 * j + 2],
                            lhsT=S_tanh[:, j * L + off : j * L + off + w],
                            rhs=v2_sb,
                            start=True,
                            stop=True,
                        )

            def expo(s):
                L = L_slots[s]
                nch = nchs[s]
                wmax = min(128, L)
                E = epool.tile(
                    [128, 64 * nchmax], BF16, name=f"E{s}", tag="E"
                )
                E_sbs[s] = E
                nc.scalar.activation(
                    E[0:wmax, 0 : 64 * nch], sc_pss[s][0:wmax, 0 : 64 * nch], EXP
                )

            def outmm(s):
                L = L_slots[s]
                nch = nchs[s]
                E = E_sbs[s]
                out_ps = oup.tile([TQ, H + 1], F32, name=f"ops{s}", tag="ou")
                out_pss[s] = out_ps
                for ci in range(nch):
                    off = 128 * ci
                    w = min(128, L - off)
                    kno = knoff[s] + ci * KXC
                    nc.tensor.matmul(
                        out_ps,
                        lhsT=E[0:w, 64 * ci : 64 * ci + 64],
                        rhs=kn_sb[0:w, kno : kno + H + 1],
                        start=(ci == 0),
                        stop=(ci == nch - 1),
                    )

            def norm(s):
                out_ps = out_pss[s]
                recip = opool.tile([TQ, 1], F32, name=f"rc{s}", tag="rc")
                nc.vector.reciprocal(recip, out_ps[:, H : H + 1])
                out_sb = opool.tile([TQ, H], F32, name=f"osb{s}", tag="osb")
                nc.vector.tensor_scalar_mul(out_sb, out_ps[:, 0:H], recip)
                nc.sync.dma_start(out=o_d[s], in_=out_sb)

            sp = _SLOT_SPLITS

            # ---- hand-interleaved schedule.
            # ACT queue: W,T0*,T1a,E0,T1b,T1c,T2a,E1,T2b,T3a,E2,T3b,E3
            # DVE queue: adds in slot order; kpb2/qp2 copies just in time;
            #            norms at the end.
            # PE queue:  projections, scores0..3 with outmms interleaved.
            for j0, j1 in sp[0]:
                adds(0, j0, j1)
                tanh(0, j0, j1)
            kproj_dve(1, kps1)
            scores(0)
            kps2 = kproj_mm(2)
            adds(1, *sp[1][0])
            tanh(1, *sp[1][0])
            expo(0)
            adds(1, *sp[1][1])
            tanh(1, *sp[1][1])
            qproj_dve(1, qps1)
            adds(1, *sp[1][2])
            tanh(1, *sp[1][2])
            kproj_dve(2, kps2)
            scores(1)
            kps3 = kproj_mm(3)
            outmm(0)
            adds(2, *sp[2][0])
            tanh(2, *sp[2][0])
            expo(1)
            adds(2, *sp[2][1])
            tanh(2, *sp[2][1])
            kproj_dve(3, kps3)
            scores(2)
            outmm(1)
            adds(3, *sp[3][0])
            tanh(3, *sp[3][0])
            expo(2)
            adds(3, *sp[3][1])
            tanh(3, *sp[3][1])
            scores(3)
            outmm(2)
            expo(3)
            outmm(3)
            for s in range(SLOTS):
                norm(s)

    nc.compile()
    return nc


def _get_prog(L_slots):
    if L_slots not in _prog_cache:
        _prog_cache[L_slots] = _build(L_slots)
    return _prog_cache[L_slots]


def _plan(seq_len_flat):
    sl = np.asarray(seq_len_flat).reshape(-1).astype(np.int64)
    order = np.argsort(-sl, kind="stable")
    # groups by descending length; compute order: [2nd-shortest grp? ->
    # actually: 3rd-longest, longest, 2nd-longest, shortest]
    grp = [order[NCORES * r : NCORES * (r + 1)] for r in range(SLOTS)]
    slot_of_rank = [1, 2, 0, 3]  # rank r (0=longest) -> slot index
    assign = np.zeros((NCORES, SLOTS), dtype=np.int64)
    L_slots = [0] * SLOTS
    for r in range(SLOTS):
        s = slot_of_rank[r]
        assign[:, s] = grp[r]
        L = int(max(1, sl[grp[r]].max()))
        L_slots[s] = min(TK, _roundup(L, 8))
    return tuple(L_slots), assign, sl


def _make_in_maps(queries, keys, sl, assign, W_q, W_k, v, b, L_slots):
    W_q = np.asarray(W_q, np.float32)
    W_k = np.asarray(W_k, np.float32)
    vv = np.asarray(v, np.float32).reshape(-1)
    bb = np.asarray(b, np.float32).reshape(-1)
    nchs = [_nch(L) for L in L_slots]
    KT = sum(L_slots)
    KN = sum(nchs) * KXC
    CX = 260 + 256

    base = np.zeros((128, CX), np.float32)
    base[0:H, 0:H] = W_q
    base[H:128, H:128] = W_q
    base[0:H, 128 : 128 + H] = W_k
    base[0:H, 128 + H : 256] = W_k
    base[0:H, 256] = bb
    base[H:128, 256] = bb
    base[0:H, 257] = vv
    base[H:128, 258] = vv

    in_maps = []
    for c in range(NCORES):
        cx = base.copy()
        for g in range(2):
            for half in range(2):
                s = 2 * g + half
                q = queries[assign[c, s]]  # [64, 64]
                up = np.repeat(q[0::2], 2, axis=0).T  # [64h, 64cols]
                lo = np.repeat(q[1::2], 2, axis=0).T
                c0 = 260 + 128 * g + 64 * half
                cx[0:H, c0 : c0 + 64] = up
                cx[H:128, c0 : c0 + 64] = lo
        ktall = np.zeros((H, KT), np.float32)
        knall = np.zeros((128, KN), np.float32)
        kto = kno = 0
        for s, L in enumerate(L_slots):
            b_i = assign[c, s]
            kk = keys[b_i]  # [256, 64]
            lv = int(min(sl[b_i], L))
            ktall[:, kto : kto + lv] = kk[0:lv].T
            for ci in range(nchs[s]):
                off = 128 * ci
                w = min(128, L - off)
                vw = max(0, min(lv - off, w))
                if vw > 0:
                    knall[0:vw, kno : kno + H] = kk[off : off + vw]
                    knall[0:vw, kno + H] = 1.0
                kno += KXC
            kto += L
        in_maps.append(
            {
                "cx": cx.astype(np_bf16),
                "ktall": ktall.astype(np_bf16),
                "knall": knall.astype(np_bf16),
            }
        )
    return in_maps


def _run_spmd(nc, in_maps, trace=False, trace_kwargs=None):
    from concourse.bass_interp import get_hw_module

    old = nc.m
    nc.m = get_hw_module(nc.m)
    try:
        res = bass_utils.run_bass_kernel_spmd(
            nc,
            in_maps,
            core_ids=list(range(NCORES)),
            trace=trace,
            **(trace_kwargs or {}),
        )
    finally:
        nc.m = old
    return res


def kernel(queries, keys, seq_len, W_q, W_k, v, b, _trace=False):
    queries = np.asarray(queries, dtype=np.float32)
    keys = np.asarray(keys, dtype=np.float32)
    L_slots, assign, sl = _plan(seq_len)
    nc = _get_prog(L_slots)
    in_maps = _make_in_maps(queries, keys, sl, assign, W_q, W_k, v, b, L_slots)
    res = _run_spmd(nc, in_maps, trace=_trace)
    out = np.zeros((B, TQ, H), np.float32)
    for c in range(NCORES):
        o = res.results[c]["out"]
        for s_i, b_i in enumerate(assign[c]):
            out[b_i] = o[s_i]
    # seq_len==0 -> reference softmax degenerates to uniform over all keys
    # (all positions masked to the same NEG_PAD).
    for b_i in np.nonzero(sl == 0)[0]:
        out[b_i] = keys[b_i].mean(axis=0, keepdims=True)
    if _trace:
        kernel._last_results = res
    return out


# revision 16
# speedup vs baseline: 1.0911x; 1.0292x over previous
"""Additive (Bahdanau) attention on Trainium2, data-parallel over batch on 8 NeuronCores.

Math (per batch b):
    qp = queries @ W_q                     [Tq, H]
    kp = keys @ W_k                        [Tk, H]
    scores[q,k] = sum_h v[h] * tanh(qp[q,h] + kp[k,h] + b[h])
    masked softmax over k (k < seq_len[b]), then out = align @ keys.

Design (per core, 4 batch "slots" with compile-time key-lengths L_slots,
ordered [3rd-longest, longest, 2nd-longest, shortest] so the pipeline fills
fast and drains on the smallest slot):
  - ALL host-side layout prep in bf16: transposed keys (ktall), natural
    mask-zeroed keys||maskones chunks (knall), and a const tensor cx holding
    W_q||b rows, [W_k|W_k], v2 and the pair-duplicated transposed queries
    (qTx-even/odd with a trailing ones row) - zero device transposes.
  - qp2+b [128, 128] per slot-pair in TWO K=65 matmuls (even-query half to
    psum[0:64], odd to psum[64:128]; the 65th contraction row is ones x b,
    folding the bias) + one DVE cast to bf16. kp per slot: one matmul
    (lhsT=[W_k|W_k] fills both 64-partition halves) + one DVE cast.
  - S[h2, j*L+k] = kpb2 + qp2[:, 2j+par] via DVE tensor_tensor adds in bf16
    (2x mode), tanh on ACT (the roofline: 32*sum(L) lane-cycles @1.2GHz).
  - scores^T[k, 2j+t] via PE: lhsT = S_tanh[:, j*L+off:+w] stationary,
    rhs = v2 [128, 2] -> psum[w, 64*ci+2j:+2]; all k-chunks of a slot land
    in one PSUM tile, so ONE exp per slot (no bias: masking lives in
    knall's zeroed rows), bf16 out feeding the final matmuls directly.
  - out_un[q, h]|rowsum: PSUM-accumulated E-chunks @ knall chunks; DVE
    reciprocal + scale, DMA out.
  - tanh j-splits are sized so each slot's LAST split is small: the exp
    (which needs every j) then trails the last tanh by only a few score
    matmuls, and the ACT queue interleaves the next slot's tanh to cover
    even that.

Batches are sorted by seq_len and dealt so each core gets one batch per slot
rank; slot k-length = max over the 8 batches of that rank (rounded to 2).
All cores run the identical program on different data (SPMD).
"""

import sys

_REPO = "/opt/trn_rl_repo"
if _REPO not in sys.path:
    sys.path.insert(0, _REPO)

import numpy as np

try:
    from ml_dtypes import bfloat16 as np_bf16
except ImportError:  # pragma: no cover
    import jax.numpy as _jnp

    np_bf16 = _jnp.bfloat16

from concourse import bacc, tile
import concourse.mybir as mybir
from concourse import bass_utils

B, TQ, TK, H = 32, 64, 256, 64
NCORES = 8
SLOTS = 4
F32 = mybir.dt.float32
BF16 = mybir.dt.bfloat16
TANH = mybir.ActivationFunctionType.Tanh
EXP = mybir.ActivationFunctionType.Exp
KXC = 66  # keys | maskones | pad, per 128-row chunk of knall
QPREF = 256  # qTx for slot-pair g=1, prefixed to knall

_prog_cache: dict = {}


def _roundup(x, m):
    return ((x + m - 1) // m) * m


def _nch(L):
    return (L + 127) // 128


# per-slot tanh/adds j-splits: last split small so exp trails it closely
_SLOT_SPLITS = [
    [(0, 8), (8, 20), (20, 32)],
    [(0, 12), (12, 24), (24, 32)],
    [(0, 12), (12, 26), (26, 32)],
    [(0, 14), (14, 28), (28, 32)],
]


def _build(L_slots):
    nc = bacc.Bacc(
        "TRN2",
        target_bir_lowering=False,
        debug=False,
        enable_asserts=False,
        num_devices=NCORES,
    )
    nchs = [_nch(L) for L in L_slots]
    KT = sum(L_slots)
    KN = QPREF + sum(nchs) * KXC
    CX = 194 + 256  # wqb65 64 | wk2 128 | v2 2 | qTx-even/odd g0

    cx_d = nc.dram_tensor("cx", [128, CX], BF16, kind="ExternalInput").ap()
    kt_d = nc.dram_tensor("ktall", [H, KT], BF16, kind="ExternalInput").ap()
    kn_d = nc.dram_tensor("knall", [128, KN], BF16, kind="ExternalInput").ap()
    o_d = nc.dram_tensor("out", [SLOTS, TQ, H], F32, kind="ExternalOutput").ap()

    ktoff = [sum(L_slots[:s]) for s in range(SLOTS)]
    knoff = [QPREF + sum(nchs[:s]) * KXC for s in range(SLOTS)]
    Lmax = max(L_slots)
    nchmax = max(nchs)

    with tile.TileContext(nc) as tc:
        with (
            tc.tile_pool(name="const", bufs=1) as cpool,
            tc.tile_pool(name="qpool", bufs=2) as qpool,
            tc.tile_pool(name="kpb", bufs=2) as kpool,
            tc.tile_pool(name="spool", bufs=2) as spool,
            tc.tile_pool(name="epool", bufs=2) as epool,
            tc.tile_pool(name="opool", bufs=2) as opool,
            tc.tile_pool(name="qpp", bufs=2, space="PSUM") as qpp,
            tc.tile_pool(name="kpp", bufs=2, space="PSUM") as kpp,
            tc.tile_pool(name="scp", bufs=2, space="PSUM") as scp,
            tc.tile_pool(name="oup", bufs=2, space="PSUM") as oup,
        ):
            # tiny activation up front so the ACT table set loads early
            scr = cpool.tile([1, 2], F32, name="scr", tag="scr")
            nc.vector.memset(scr, 0.0)
            nc.scalar.activation(scr, scr, TANH)

            # ---- prefetch in three DMAs, hottest first
            cx_sb = cpool.tile([128, CX], BF16, name="cx_sb", tag="cx")
            nc.sync.dma_start(out=cx_sb, in_=cx_d)
            kt_sb = cpool.tile([H, KT], BF16, name="kt_sb", tag="kt")
            nc.sync.dma_start(out=kt_sb, in_=kt_d)
            kn_sb = cpool.tile([128, KN], BF16, name="kn_sb", tag="kn")
            nc.sync.dma_start(out=kn_sb, in_=kn_d)

            wqb65 = cx_sb[0:65, 0:64]
            wk2_sb = cx_sb[0:H, 64:192]
            v2_sb = cx_sb[:, 192:194]
            qtx = {
                (0, 0): cx_sb[0:65, 194:322],
                (0, 1): cx_sb[0:65, 322:450],
                (1, 0): kn_sb[0:65, 0:128],
                (1, 1): kn_sb[0:65, 128:256],
            }

            qp2_sb = [None, None]

            def qproj_mm(g):
                ps = qpp.tile([128, 128], F32, name=f"qpps{g}", tag="qp")
                nc.tensor.matmul(ps[0:64, :], lhsT=wqb65, rhs=qtx[(g, 0)])
                nc.tensor.matmul(ps[64:128, :], lhsT=wqb65, rhs=qtx[(g, 1)])
                return ps

            def qproj_dve(g, ps):
                qp2 = qpool.tile([128, 128], BF16, name=f"qp2_{g}", tag="qp2")
                nc.vector.tensor_copy(qp2, ps)
                qp2_sb[g] = qp2

            kpb2s = {}

            def kproj_mm(s):
                L = L_slots[s]
                ps = kpp.tile([128, Lmax], F32, name=f"kpps{s}", tag="kp")
                nc.tensor.matmul(
                    ps[:, 0:L], lhsT=wk2_sb, rhs=kt_sb[:, ktoff[s] : ktoff[s] + L]
                )
                return ps

            def kproj_dve(s, ps):
                L = L_slots[s]
                kpb2 = kpool.tile([128, Lmax], BF16, name=f"kpb2_{s}", tag="kpb2")
                nc.vector.tensor_copy(kpb2[:, 0:L], ps[:, 0:L])
                kpb2s[s] = kpb2

            # ---- per-slot stages
            S_alls, S_tanhs, sc_pss, E_sbs, out_pss = {}, {}, {}, {}, {}

            def adds(s, j0, j1):
                L = L_slots[s]
                if s not in S_alls:
                    S_alls[s] = spool.tile(
                        [128, 32 * Lmax], BF16, name=f"S{s}", tag="S"
                    )
                S_all = S_alls[s]
                kpb2 = kpb2s[s]
                qp2 = qp2_sb[s // 2]
                qoff = 64 * (s % 2)
                nj = j1 - j0
                in0 = (
                    kpb2[:, 0:L]
                    .rearrange("c (k two) -> c k two", two=2)
                    .unsqueeze(1)
                    .broadcast_to([128, nj, L // 2, 2])
                )
                in1 = (
                    qp2[:, qoff + 2 * j0 : qoff + 2 * j1]
                    .rearrange("c (j two) -> c j two", two=2)
                    .unsqueeze(2)
                    .broadcast_to([128, nj, L // 2, 2])
                )
                s_out = S_all[:, j0 * L : j1 * L].rearrange(
                    "c (j k two) -> c j k two", two=2, k=L // 2
                )
                nc.vector.tensor_add(s_out, in0, in1)

            def tanh(s, j0, j1):
                L = L_slots[s]
                if s not in S_tanhs:
                    S_tanhs[s] = spool.tile(
                        [128, 32 * Lmax], BF16, name=f"T{s}", tag="T"
                    )
                nc.scalar.activation(
                    S_tanhs[s][:, j0 * L : j1 * L],
                    S_alls[s][:, j0 * L : j1 * L],
                    TANH,
                )

            def scores(s, j0, j1):
                L = L_slots[s]
                nch = nchs[s]
                S_tanh = S_tanhs[s]
                if s not in sc_pss:
                    sc_pss[s] = scp.tile(
                        [128, 64 * nchmax], F32, name=f"sc{s}", tag="sc"
                    )
                    wl = L - 128 * (nch - 1)
                    if nch > 1 and wl < 128:
                        # exp reads the whole tile; pre-zero the last chunk's
                        # columns (its matmuls overwrite rows [0:wl])
                        nc.vector.memset(
                            sc_pss[s][:, 64 * (nch - 1) : 64 * nch], 0.0
                        )
                sc_ps = sc_pss[s]
                for ci in range(nch):
                    off = 128 * ci
                    w = min(128, L - off)
                    for j in range(j0, j1):
                        nc.tensor.matmul(
                            sc_ps[0:w, 64 * ci + 2 * j : 64 * ci + 2 * j + 2],
                            lhsT=S_tanh[:, j * L + off : j * L + off + w],
                            rhs=v2_sb,
                            start=True,
                            stop=True,
                        )

            def expo(s):
                L = L_slots[s]
                nch = nchs[s]
                wmax = min(128, L)
                E = epool.tile(
                    [128, 64 * nchmax], BF16, name=f"E{s}", tag="E"
                )
                E_sbs[s] = E
                nc.scalar.activation(
                    E[0:wmax, 0 : 64 * nch], sc_pss[s][0:wmax, 0 : 64 * nch], EXP
                )

            def outmm(s):
                L = L_slots[s]
                nch = nchs[s]
                E = E_sbs[s]
                out_ps = oup.tile([TQ, H + 1], F32, name=f"ops{s}", tag="ou")
                out_pss[s] = out_ps
                for ci in range(nch):
                    off = 128 * ci
                    w = min(128, L - off)
                    kno = knoff[s] + ci * KXC
                    nc.tensor.matmul(
                        out_ps,
                        lhsT=E[0:w, 64 * ci : 64 * ci + 64],
                        rhs=kn_sb[0:w, kno : kno + H + 1],
                        start=(ci == 0),
                        stop=(ci == nch - 1),
                    )

            def norm(s):
                out_ps = out_pss[s]
                recip = opool.tile([TQ, 1], F32, name=f"rc{s}", tag="rc")
                nc.vector.reciprocal(recip, out_ps[:, H : H + 1])
                out_sb = opool.tile([TQ, H], F32, name=f"osb{s}", tag="osb")
                nc.vector.tensor_scalar_mul(out_sb, out_ps[:, 0:H], recip)
                nc.sync.dma_start(out=o_d[s], in_=out_sb)

            sp = _SLOT_SPLITS

            # ---- hand-interleaved schedule.
            # ACT: W,T0abc,T1a,E0,T1b,T1c,T2a,E1,T2b,T3a,T2c,T3b,E2,T3c,E3
            # DVE: casts just-in-time, adds in ACT order, norms at the end.
            # PE:  projections early, scores in j-split order, outmms asap.
            qps0 = qproj_mm(0)
            qproj_dve(0, qps0)
            kps0 = kproj_mm(0)
            kproj_dve(0, kps0)
            kps1 = kproj_mm(1)

            for j0, j1 in sp[0]:
                adds(0, j0, j1)
                tanh(0, j0, j1)
                scores(0, j0, j1)
            kproj_dve(1, kps1)
            qps1 = qproj_mm(1)
            kps2 = kproj_mm(2)

            adds(1, *sp[1][0])
            tanh(1, *sp[1][0])
            expo(0)
            scores(1, *sp[1][0])
            adds(1, *sp[1][1])
            tanh(1, *sp[1][1])
            scores(1, *sp[1][1])
            qproj_dve(1, qps1)
            adds(1, *sp[1][2])
            tanh(1, *sp[1][2])
            scores(1, *sp[1][2])
            kproj_dve(2, kps2)
            kps3 = kproj_mm(3)
            outmm(0)

            adds(2, *sp[2][0])
            tanh(2, *sp[2][0])
            expo(1)
            scores(2, *sp[2][0])
            adds(2, *sp[2][1])
            tanh(2, *sp[2][1])
            scores(2, *sp[2][1])
            kproj_dve(3, kps3)
            outmm(1)

            adds(3, *sp[3][0])
            tanh(3, *sp[3][0])
            adds(2, *sp[2][2])
            tanh(2, *sp[2][2])
            scores(2, *sp[2][2])
            adds(3, *sp[3][1])
            tanh(3, *sp[3][1])
            scores(3, *sp[3][0])
            scores(3, *sp[3][1])
            expo(2)
            adds(3, *sp[3][2])
            tanh(3, *sp[3][2])
            scores(3, *sp[3][2])
            outmm(2)
            expo(3)
            outmm(3)
            for s in range(SLOTS):
                norm(s)

    nc.compile()
    return nc


def _get_prog(L_slots):
    if L_slots not in _prog_cache:
        _prog_cache[L_slots] = _build(L_slots)
    return _prog_cache[L_slots]


def _plan(seq_len_flat):
    sl = np.asarray(seq_len_flat).reshape(-1).astype(np.int64)
    order = np.argsort(-sl, kind="stable")
    grp = [order[NCORES * r : NCORES * (r + 1)] for r in range(SLOTS)]
    slot_of_rank = [1, 2, 0, 3]  # rank r (0=longest) -> slot index
    assign = np.zeros((NCORES, SLOTS), dtype=np.int64)
    L_slots = [0] * SLOTS
    for r in range(SLOTS):
        s = slot_of_rank[r]
        assign[:, s] = grp[r]
        L = int(max(1, sl[grp[r]].max()))
        L_slots[s] = min(TK, _roundup(L, 2))
    return tuple(L_slots), assign, sl


def _make_in_maps(queries, keys, sl, assign, W_q, W_k, v, b, L_slots):
    W_q = np.asarray(W_q, np.float32)
    W_k = np.asarray(W_k, np.float32)
    vv = np.asarray(v, np.float32).reshape(-1)
    bb = np.asarray(b, np.float32).reshape(-1)
    nchs = [_nch(L) for L in L_slots]
    KT = sum(L_slots)
    KN = QPREF + sum(nchs) * KXC
    CX = 194 + 256

    base = np.zeros((128, CX), np.float32)
    base[0:H, 0:H] = W_q
    base[H, 0:H] = bb
    base[0:H, 64 : 64 + H] = W_k
    base[0:H, 64 + H : 192] = W_k
    base[0:H, 192] = vv
    base[H:128, 193] = vv

    def qtx_cols(q):
        # [65, 64]: row h = queries[2*(m//2) (+1 for odd), h].T; row 64 = 1
        up = np.zeros((65, 64), np.float32)
        lo = np.zeros((65, 64), np.float32)
        up[0:H] = np.repeat(q[0::2], 2, axis=0).T
        lo[0:H] = np.repeat(q[1::2], 2, axis=0).T
        up[H] = 1.0
        lo[H] = 1.0
        return up, lo

    in_maps = []
    for c in range(NCORES):
        cx = base.copy()
        knall = np.zeros((128, KN), np.float32)
        for g in range(2):
            for half in range(2):
                s = 2 * g + half
                up, lo = qtx_cols(queries[assign[c, s]])
                if g == 0:
                    cx[0:65, 194 + 64 * half : 258 + 64 * half] = up
                    cx[0:65, 322 + 64 * half : 386 + 64 * half] = lo
                else:
                    knall[0:65, 64 * half : 64 * half + 64] = up
                    knall[0:65, 128 + 64 * half : 192 + 64 * half] = lo
        ktall = np.zeros((H, KT), np.float32)
        kto, kno = 0, QPREF
        for s, L in enumerate(L_slots):
            b_i = assign[c, s]
            kk = keys[b_i]  # [256, 64]
            lv = int(min(sl[b_i], L))
            ktall[:, kto : kto + lv] = kk[0:lv].T
            for ci in range(nchs[s]):
                off = 128 * ci
                w = min(128, L - off)
                vw = max(0, min(lv - off, w))
                if vw > 0:
                    knall[0:vw, kno : kno + H] = kk[off : off + vw]
                    knall[0:vw, kno + H] = 1.0
                kno += KXC
            kto += L
        in_maps.append(
            {
                "cx": cx.astype(np_bf16),
                "ktall": ktall.astype(np_bf16),
                "knall": knall.astype(np_bf16),
            }
        )
    return in_maps


def _run_spmd(nc, in_maps, trace=False, trace_kwargs=None):
    from concourse.bass_interp import get_hw_module

    old = nc.m
    nc.m = get_hw_module(nc.m)
    try:
        res = bass_utils.run_bass_kernel_spmd(
            nc,
            in_maps,
            core_ids=list(range(NCORES)),
            trace=trace,
            **(trace_kwargs or {}),
        )
    finally:
        nc.m = old
    return res


def kernel(queries, keys, seq_len, W_q, W_k, v, b, _trace=False):
    queries = np.asarray(queries, dtype=np.float32)
    keys = np.asarray(keys, dtype=np.float32)
    L_slots, assign, sl = _plan(seq_len)
    nc = _get_prog(L_slots)
    in_maps = _make_in_maps(queries, keys, sl, assign, W_q, W_k, v, b, L_slots)
    res = _run_spmd(nc, in_maps, trace=_trace)
    out = np.zeros((B, TQ, H), np.float32)
    for c in range(NCORES):
        o = res.results[c]["out"]
        for s_i, b_i in enumerate(assign[c]):
            out[b_i] = o[s_i]
    # seq_len==0 -> reference softmax degenerates to uniform over all keys
    # (all positions masked to the same NEG_PAD).
    for b_i in np.nonzero(sl == 0)[0]:
        out[b_i] = keys[b_i].mean(axis=0, keepdims=True)
    if _trace:
        kernel._last_results = res
    return out


# revision 17
# speedup vs baseline: 1.1452x; 1.0496x over previous
"""Additive (Bahdanau) attention on Trainium2, data-parallel over batch on 8 NeuronCores.

Math (per batch b):
    qp = queries @ W_q                     [Tq, H]
    kp = keys @ W_k                        [Tk, H]
    scores[q,k] = sum_h v[h] * tanh(qp[q,h] + kp[k,h] + b[h])
    masked softmax over k (k < seq_len[b]), then out = align @ keys.

Design (per core, 4 batch "slots" with compile-time key-lengths L_slots,
ordered [3rd-longest, longest, 2nd-longest, shortest] so the pipeline fills
fast and drains on the smallest slot):
  - ALL host-side layout prep in bf16: transposed keys (ktall), natural
    mask-zeroed keys||maskones chunks (knall), and a const tensor cx holding
    W_q||b rows, [W_k|W_k], v2 and the pair-duplicated transposed queries
    (qTx-even/odd with a trailing ones row) - zero device transposes.
  - qp2+b [128, 128] per slot-pair in TWO K=65 matmuls (even-query half to
    psum[0:64], odd to psum[64:128]; the 65th contraction row is ones x b,
    folding the bias) + one DVE cast to bf16. kp per slot: one matmul
    (lhsT=[W_k|W_k] fills both 64-partition halves) + one DVE cast.
  - S[h2, j*L+k] = kpb2 + qp2[:, 2j+par] via DVE tensor_tensor adds in bf16
    (2x mode), tanh on ACT (the roofline: 32*sum(L) lane-cycles @1.2GHz).
  - scores^T[k, 2j+t] via PE: lhsT = S_tanh[:, j*L+off:+w] stationary,
    rhs = v2 [128, 2] -> psum[w, 64*ci+2j:+2]; all k-chunks of a slot land
    in one PSUM tile, so ONE exp per slot (no bias: masking lives in
    knall's zeroed rows), bf16 out feeding the final matmuls directly.
  - out_un[q, h]|rowsum: PSUM-accumulated E-chunks @ knall chunks; DVE
    reciprocal + scale, DMA out.
  - tanh j-splits are sized so each slot's LAST split is small: the exp
    (which needs every j) then trails the last tanh by only a few score
    matmuls, and the ACT queue interleaves the next slot's tanh to cover
    even that.

Batches are sorted by seq_len and dealt so each core gets one batch per slot
rank; slot k-length = max over the 8 batches of that rank (rounded to 2).
All cores run the identical program on different data (SPMD).
"""

import sys

_REPO = "/opt/trn_rl_repo"
if _REPO not in sys.path:
    sys.path.insert(0, _REPO)

import numpy as np

try:
    from ml_dtypes import bfloat16 as np_bf16
except ImportError:  # pragma: no cover
    import jax.numpy as _jnp

    np_bf16 = _jnp.bfloat16

from concourse import bacc, tile
import concourse.mybir as mybir
from concourse import bass_utils

B, TQ, TK, H = 32, 64, 256, 64
NCORES = 8
SLOTS = 4
F32 = mybir.dt.float32
BF16 = mybir.dt.bfloat16
TANH = mybir.ActivationFunctionType.Tanh
EXP = mybir.ActivationFunctionType.Exp
KXC = 66  # keys | maskones | pad, per 128-row chunk of knall
QPREF = 256  # qTx for slot-pair g=1, prefixed to knall

_prog_cache: dict = {}


def _roundup(x, m):
    return ((x + m - 1) // m) * m


def _nch(L):
    return (L + 127) // 128


# per-slot tanh/adds j-splits: last split small so exp trails it closely
_SLOT_SPLITS = [
    [(0, 8), (8, 20), (20, 32)],
    [(0, 12), (12, 24), (24, 32)],
    [(0, 12), (12, 26), (26, 32)],
    [(0, 14), (14, 28), (28, 32)],
]


def _build(L_slots):
    nc = bacc.Bacc(
        "TRN2",
        target_bir_lowering=False,
        debug=False,
        enable_asserts=False,
        num_devices=NCORES,
    )
    nchs = [_nch(L) for L in L_slots]
    KT = sum(L_slots)
    KN = QPREF + sum(nchs) * KXC
    CX = 194 + 256  # wqb65 64 | wk2 128 | v2 2 | qTx-even/odd g0

    cx_d = nc.dram_tensor("cx", [128, CX], BF16, kind="ExternalInput").ap()
    kt_d = nc.dram_tensor("ktall", [H, KT], BF16, kind="ExternalInput").ap()
    kn_d = nc.dram_tensor("knall", [128, KN], BF16, kind="ExternalInput").ap()
    o_d = nc.dram_tensor("out", [SLOTS, TQ, H], F32, kind="ExternalOutput").ap()

    ktoff = [sum(L_slots[:s]) for s in range(SLOTS)]
    knoff = [QPREF + sum(nchs[:s]) * KXC for s in range(SLOTS)]
    Lmax = max(L_slots)
    nchmax = max(nchs)

    with tile.TileContext(nc) as tc:
        with (
            tc.tile_pool(name="const", bufs=1) as cpool,
            tc.tile_pool(name="qpool", bufs=2) as qpool,
            tc.tile_pool(name="kpb", bufs=2) as kpool,
            tc.tile_pool(name="spool", bufs=2) as spool,
            tc.tile_pool(name="epool", bufs=2) as epool,
            tc.tile_pool(name="opool", bufs=2) as opool,
            tc.tile_pool(name="qpp", bufs=2, space="PSUM") as qpp,
            tc.tile_pool(name="kpp", bufs=2, space="PSUM") as kpp,
            tc.tile_pool(name="scp", bufs=2, space="PSUM") as scp,
            tc.tile_pool(name="oup", bufs=2, space="PSUM") as oup,
        ):
            # ---- prefetch in three DMAs on three different engines so the
            # three hardware queues stream in parallel (hottest via ACT,
            # whose BSP startup finishes first)
            cx_sb = cpool.tile([128, CX], BF16, name="cx_sb", tag="cx")
            nc.scalar.dma_start(out=cx_sb, in_=cx_d)
            kt_sb = cpool.tile([H, KT], BF16, name="kt_sb", tag="kt")
            nc.sync.dma_start(out=kt_sb, in_=kt_d)
            kn_sb = cpool.tile([128, KN], BF16, name="kn_sb", tag="kn")
            nc.gpsimd.dma_start(out=kn_sb, in_=kn_d)

            # tiny activation so the ACT table set loads during the DMA wait
            scr = cpool.tile([1, 2], F32, name="scr", tag="scr")
            nc.vector.memset(scr, 0.0)
            nc.scalar.activation(scr, scr, TANH)

            wqb65 = cx_sb[0:65, 0:64]
            wk2_sb = cx_sb[0:H, 64:192]
            v2_sb = cx_sb[:, 192:194]
            qtx = {
                (0, 0): cx_sb[0:65, 194:322],
                (0, 1): cx_sb[0:65, 322:450],
                (1, 0): kn_sb[0:65, 0:128],
                (1, 1): kn_sb[0:65, 128:256],
            }

            qp2_sb = [None, None]

            def qproj_mm(g):
                ps = qpp.tile([128, 128], F32, name=f"qpps{g}", tag="qp")
                nc.tensor.matmul(ps[0:64, :], lhsT=wqb65, rhs=qtx[(g, 0)])
                nc.tensor.matmul(ps[64:128, :], lhsT=wqb65, rhs=qtx[(g, 1)])
                return ps

            def qproj_dve(g, ps):
                qp2 = qpool.tile([128, 128], BF16, name=f"qp2_{g}", tag="qp2")
                nc.vector.tensor_copy(qp2, ps)
                qp2_sb[g] = qp2

            kpb2s = {}

            def kproj_mm(s):
                L = L_slots[s]
                ps = kpp.tile([128, Lmax], F32, name=f"kpps{s}", tag="kp")
                nc.tensor.matmul(
                    ps[:, 0:L], lhsT=wk2_sb, rhs=kt_sb[:, ktoff[s] : ktoff[s] + L]
                )
                return ps

            def kproj_dve(s, ps):
                L = L_slots[s]
                kpb2 = kpool.tile([128, Lmax], BF16, name=f"kpb2_{s}", tag="kpb2")
                nc.vector.tensor_copy(kpb2[:, 0:L], ps[:, 0:L])
                kpb2s[s] = kpb2

            # ---- per-slot stages
            S_alls, S_tanhs, sc_pss, E_sbs, out_pss = {}, {}, {}, {}, {}

            def adds(s, j0, j1):
                L = L_slots[s]
                if s not in S_alls:
                    S_alls[s] = spool.tile(
                        [128, 32 * Lmax], BF16, name=f"S{s}", tag="S"
                    )
                S_all = S_alls[s]
                kpb2 = kpb2s[s]
                qp2 = qp2_sb[s // 2]
                qoff = 64 * (s % 2)
                nj = j1 - j0
                in0 = (
                    kpb2[:, 0:L]
                    .rearrange("c (k two) -> c k two", two=2)
                    .unsqueeze(1)
                    .broadcast_to([128, nj, L // 2, 2])
                )
                in1 = (
                    qp2[:, qoff + 2 * j0 : qoff + 2 * j1]
                    .rearrange("c (j two) -> c j two", two=2)
                    .unsqueeze(2)
                    .broadcast_to([128, nj, L // 2, 2])
                )
                s_out = S_all[:, j0 * L : j1 * L].rearrange(
                    "c (j k two) -> c j k two", two=2, k=L // 2
                )
                nc.vector.tensor_add(s_out, in0, in1)

            def tanh(s, j0, j1):
                L = L_slots[s]
                if s not in S_tanhs:
                    S_tanhs[s] = spool.tile(
                        [128, 32 * Lmax], BF16, name=f"T{s}", tag="T"
                    )
                nc.scalar.activation(
                    S_tanhs[s][:, j0 * L : j1 * L],
                    S_alls[s][:, j0 * L : j1 * L],
                    TANH,
                )

            def scores(s, j0, j1):
                L = L_slots[s]
                nch = nchs[s]
                S_tanh = S_tanhs[s]
                if s not in sc_pss:
                    sc_pss[s] = scp.tile(
                        [128, 64 * nchmax], F32, name=f"sc{s}", tag="sc"
                    )
                    wl = L - 128 * (nch - 1)
                    if nch > 1 and wl < 128:
                        # exp reads the whole tile; pre-zero the last chunk's
                        # columns (its matmuls overwrite rows [0:wl])
                        nc.vector.memset(
                            sc_pss[s][:, 64 * (nch - 1) : 64 * nch], 0.0
                        )
                sc_ps = sc_pss[s]
                for ci in range(nch):
                    off = 128 * ci
                    w = min(128, L - off)
                    for j in range(j0, j1):
                        nc.tensor.matmul(
                            sc_ps[0:w, 64 * ci + 2 * j : 64 * ci + 2 * j + 2],
                            lhsT=S_tanh[:, j * L + off : j * L + off + w],
                            rhs=v2_sb,
                            start=True,
                            stop=True,
                        )

            def expo(s):
                L = L_slots[s]
                nch = nchs[s]
                wmax = min(128, L)
                E = epool.tile(
                    [128, 64 * nchmax], BF16, name=f"E{s}", tag="E"
                )
                E_sbs[s] = E
                nc.scalar.activation(
                    E[0:wmax, 0 : 64 * nch], sc_pss[s][0:wmax, 0 : 64 * nch], EXP
                )

            def outmm(s):
                L = L_slots[s]
                nch = nchs[s]
                E = E_sbs[s]
                out_ps = oup.tile([TQ, H + 1], F32, name=f"ops{s}", tag="ou")
                out_pss[s] = out_ps
                for ci in range(nch):
                    off = 128 * ci
                    w = min(128, L - off)
                    kno = knoff[s] + ci * KXC
                    nc.tensor.matmul(
                        out_ps,
                        lhsT=E[0:w, 64 * ci : 64 * ci + 64],
                        rhs=kn_sb[0:w, kno : kno + H + 1],
                        start=(ci == 0),
                        stop=(ci == nch - 1),
                    )

            def norm(s):
                out_ps = out_pss[s]
                recip = opool.tile([TQ, 1], F32, name=f"rc{s}", tag="rc")
                nc.vector.reciprocal(recip, out_ps[:, H : H + 1])
                out_sb = opool.tile([TQ, H], F32, name=f"osb{s}", tag="osb")
                nc.vector.tensor_scalar_mul(out_sb, out_ps[:, 0:H], recip)
                nc.sync.dma_start(out=o_d[s], in_=out_sb)

            sp = _SLOT_SPLITS

            # ---- hand-interleaved schedule.
            # ACT: W,T0abc,T1a,E0,T1b,T1c,T2a,E1,T2b,T3a,T2c,T3b,E2,T3c,E3
            # DVE: casts just-in-time, adds in ACT order, norms at the end.
            # PE:  projections early, scores in j-split order, outmms asap.
            qps0 = qproj_mm(0)
            qproj_dve(0, qps0)
            kps0 = kproj_mm(0)
            kproj_dve(0, kps0)
            kps1 = kproj_mm(1)

            for j0, j1 in sp[0]:
                adds(0, j0, j1)
                tanh(0, j0, j1)
                scores(0, j0, j1)
            kproj_dve(1, kps1)
            qps1 = qproj_mm(1)
            kps2 = kproj_mm(2)

            adds(1, *sp[1][0])
            tanh(1, *sp[1][0])
            expo(0)
            scores(1, *sp[1][0])
            adds(1, *sp[1][1])
            tanh(1, *sp[1][1])
            scores(1, *sp[1][1])
            qproj_dve(1, qps1)
            adds(1, *sp[1][2])
            tanh(1, *sp[1][2])
            scores(1, *sp[1][2])
            kproj_dve(2, kps2)
            kps3 = kproj_mm(3)
            outmm(0)

            adds(2, *sp[2][0])
            tanh(2, *sp[2][0])
            expo(1)
            scores(2, *sp[2][0])
            adds(2, *sp[2][1])
            tanh(2, *sp[2][1])
            scores(2, *sp[2][1])
            kproj_dve(3, kps3)
            outmm(1)

            adds(3, *sp[3][0])
            tanh(3, *sp[3][0])
            adds(2, *sp[2][2])
            tanh(2, *sp[2][2])
            scores(2, *sp[2][2])
            adds(3, *sp[3][1])
            tanh(3, *sp[3][1])
            scores(3, *sp[3][0])
            scores(3, *sp[3][1])
            expo(2)
            adds(3, *sp[3][2])
            tanh(3, *sp[3][2])
            scores(3, *sp[3][2])
            outmm(2)
            expo(3)
            outmm(3)
            for s in range(SLOTS):
                norm(s)

    nc.compile()
    return nc


def _get_prog(L_slots):
    if L_slots not in _prog_cache:
        _prog_cache[L_slots] = _build(L_slots)
    return _prog_cache[L_slots]


def _plan(seq_len_flat):
    sl = np.asarray(seq_len_flat).reshape(-1).astype(np.int64)
    order = np.argsort(-sl, kind="stable")
    grp = [order[NCORES * r : NCORES * (r + 1)] for r in range(SLOTS)]
    slot_of_rank = [1, 2, 0, 3]  # rank r (0=longest) -> slot index
    assign = np.zeros((NCORES, SLOTS), dtype=np.int64)
    L_slots = [0] * SLOTS
    for r in range(SLOTS):
        s = slot_of_rank[r]
        assign[:, s] = grp[r]
        L = int(max(1, sl[grp[r]].max()))
        L_slots[s] = min(TK, _roundup(L, 2))
    return tuple(L_slots), assign, sl


def _make_in_maps(queries, keys, sl, assign, W_q, W_k, v, b, L_slots):
    W_q = np.asarray(W_q, np.float32)
    W_k = np.asarray(W_k, np.float32)
    vv = np.asarray(v, np.float32).reshape(-1)
    bb = np.asarray(b, np.float32).reshape(-1)
    nchs = [_nch(L) for L in L_slots]
    KT = sum(L_slots)
    KN = QPREF + sum(nchs) * KXC
    CX = 194 + 256

    base = np.zeros((128, CX), np.float32)
    base[0:H, 0:H] = W_q
    base[H, 0:H] = bb
    base[0:H, 64 : 64 + H] = W_k
    base[0:H, 64 + H : 192] = W_k
    base[0:H, 192] = vv
    base[H:128, 193] = vv

    def qtx_cols(q):
        # [65, 64]: row h = queries[2*(m//2) (+1 for odd), h].T; row 64 = 1
        up = np.zeros((65, 64), np.float32)
        lo = np.zeros((65, 64), np.float32)
        up[0:H] = np.repeat(q[0::2], 2, axis=0).T
        lo[0:H] = np.repeat(q[1::2], 2, axis=0).T
        up[H] = 1.0
        lo[H] = 1.0
        return up, lo

    in_maps = []
    for c in range(NCORES):
        cx = base.copy()
        knall = np.zeros((128, KN), np.float32)
        for g in range(2):
            for half in range(2):
                s = 2 * g + half
                up, lo = qtx_cols(queries[assign[c, s]])
                if g == 0:
                    cx[0:65, 194 + 64 * half : 258 + 64 * half] = up
                    cx[0:65, 322 + 64 * half : 386 + 64 * half] = lo
                else:
                    knall[0:65, 64 * half : 64 * half + 64] = up
                    knall[0:65, 128 + 64 * half : 192 + 64 * half] = lo
        ktall = np.zeros((H, KT), np.float32)
        kto, kno = 0, QPREF
        for s, L in enumerate(L_slots):
            b_i = assign[c, s]
            kk = keys[b_i]  # [256, 64]
            lv = int(min(sl[b_i], L))
            ktall[:, kto : kto + lv] = kk[0:lv].T
            for ci in range(nchs[s]):
                off = 128 * ci
                w = min(128, L - off)
                vw = max(0, min(lv - off, w))
                if vw > 0:
                    knall[0:vw, kno : kno + H] = kk[off : off + vw]
                    knall[0:vw, kno + H] = 1.0
                kno += KXC
            kto += L
        in_maps.append(
            {
                "cx": cx.astype(np_bf16),
                "ktall": ktall.astype(np_bf16),
                "knall": knall.astype(np_bf16),
            }
        )
    return in_maps


def _run_spmd(nc, in_maps, trace=False, trace_kwargs=None):
    from concourse.bass_interp import get_hw_module

    old = nc.m
    nc.m = get_hw_module(nc.m)
    try:
        res = bass_utils.run_bass_kernel_spmd(
            nc,
            in_maps,
            core_ids=list(range(NCORES)),
            trace=trace,
            **(trace_kwargs or {}),
        )
    finally:
        nc.m = old
    return res


def kernel(queries, keys, seq_len, W_q, W_k, v, b, _trace=False):
    queries = np.asarray(queries, dtype=np.float32)
    keys = np.asarray(keys, dtype=np.float32)
    L_slots, assign, sl = _plan(seq_len)
    nc = _get_prog(L_slots)
    in_maps = _make_in_maps(queries, keys, sl, assign, W_q, W_k, v, b, L_slots)
    res = _run_spmd(nc, in_maps, trace=_trace)
    out = np.zeros((B, TQ, H), np.float32)
    for c in range(NCORES):
        o = res.results[c]["out"]
        for s_i, b_i in enumerate(assign[c]):
            out[b_i] = o[s_i]
    # seq_len==0 -> reference softmax degenerates to uniform over all keys
    # (all positions masked to the same NEG_PAD).
    for b_i in np.nonzero(sl == 0)[0]:
        out[b_i] = keys[b_i].mean(axis=0, keepdims=True)
    if _trace:
        kernel._last_results = res
    return out
